# revision 17
# baseline (speedup 1.0000x reference)
"""Multi-head self-attention (RoPE, causal) Trainium2 Bass kernel.

Sharding: 8 cores = 4 batches x 2 head-groups (8 heads each).
Per core the device kernel computes, for its batch b and head-group g:
    q/k/v = x_b @ W*[:, g] (+bias), RoPE on q/k, causal softmax attention,
    partial out-projection y @ Wo[g]  -> [2048, 1024] (bf16).
Host sums the two head-group partials per batch and adds bo.

RoPE runs in a per-head permuted basis (evens then odds) prepared on the
host by permuting Wq/Wk columns: rotate-half becomes a swap of contiguous
32-partition halves (SBUF->SBUF DMAs) with the signs folded into the
sin table.  Scores are invariant to the shared q/k basis permutation.

Device layouts (per core):
    xT   [128, 8, 2048] bf16  x_b transposed (host-prepped)
    qT'  [128, 4, 2048] bf16  roped queries, head-pair dims on partitions
    kT'  [128, 8, 2048] bf16  roped keys, per head zero-padded to 128 rows
    v    [128, 16kb, 8h, 128] bf16; even heads: dims at cols 0-63, ones
                              col 64; odd heads: ones col 0, dims at
                              cols 64-127 (psum rows then match yT rows)
    att  [128, 17408] bf16    exp(scores^T) per head, causal-trapezoid packed
    yp   [128, 1024] psum     flipped AV: lhsT=v (stationary), rhs=att
                              (streaming) -> yT directly + denom row
    yT   [128, 4, 2048] bf16  normalized attention outputs for out-proj
"""

import os
import sys

import numpy as np

for _p in ("/opt/trn_rl_repo", "/root/.axon_site/_ro/trn_rl_repo"):
    if os.path.isdir(_p) and _p not in sys.path:
        sys.path.append(_p)

import ml_dtypes  # noqa: E402

BF16 = ml_dtypes.bfloat16

B, S, D_MODEL = 4, 2048, 1024
N_HEADS, HEAD_DIM = 16, 64
N_CORES = 8
HG = 2                      # head groups
HPC = N_HEADS // HG         # heads per core = 8
DL = HPC * HEAD_DIM         # local dims per core = 512
SCALE = HEAD_DIM ** -0.5
P = 128
KC = D_MODEL // P           # k chunks in projections = 8
MB = DL // P                # m blocks (head pairs) = 4
NKB = S // P                # 128-row blocks of sequence = 16
HH = HEAD_DIM // 2          # 32
SH = S // 2                 # 1024, attention processed in q halves
CW = 1024                   # score psum chunk width

# packed causal-trapezoid offsets: att row-block ck covers q in [128*ck, S)
ATT_OFF = [0] * (NKB + 1)
for _ck in range(NKB):
    ATT_OFF[_ck + 1] = ATT_OFF[_ck] + (S - P * _ck)
ATT_TOT = ATT_OFF[NKB]      # 17408

_CACHE = {}
STAGE_OF = {}


def _tag(inst, stage):
    try:
        STAGE_OF[str(inst.ins.name)] = stage
    except Exception:
        pass
    return inst


def _build_bass():
    import concourse.tile as tile
    from concourse import bacc, mybir

    dt = mybir.dt
    nc = bacc.Bacc("TRN2", target_bir_lowering=False, debug=False)

    def din(name, shape, d=dt.bfloat16):
        return nc.dram_tensor(name, shape, d, kind="ExternalInput").ap()

    xT_d = din("xT", [D_MODEL, S])
    wq_d = din("wq", [D_MODEL, DL])
    wk_d = din("wk", [D_MODEL, DL])
    wv_d = din("wv", [D_MODEL, DL])
    wo_d = din("wo", [DL, D_MODEL])
    bq_d = din("bqT", [P, MB], dt.float32)
    bk_d = din("bkT", [P, MB], dt.float32)
    bv_d = din("bv", [1, DL])
    cos_d = din("cosT", [P, S])
    sin_d = din("sinT", [P, S])          # sign-folded (pi-basis)
    tri_d = din("tri", [P, P])
    o_d = nc.dram_tensor("o", [S, D_MODEL], dt.bfloat16,
                         kind="ExternalOutput").ap()

    FExp = mybir.ActivationFunctionType.Exp
    MUL = mybir.AluOpType.mult
    ADD = mybir.AluOpType.add

    with tile.TileContext(nc) as tc:
        with (
            tc.tile_pool(name="persist", bufs=1) as persist,
            tc.tile_pool(name="small", bufs=1) as small,
        ):
            qTf = persist.tile([P, MB, S], dt.bfloat16, tag="qTf")
            kTf = persist.tile([P, HPC, S], dt.bfloat16, tag="kTf")
            nc.vector.memset(kTf, 0.0)
            v_sb = persist.tile([P, NKB, HPC, P], dt.bfloat16, tag="v_sb")
            nc.vector.memset(v_sb, 0.0)
            yT_all = persist.tile([P, MB, S], dt.bfloat16, tag="yT")
            wo_sb = persist.tile([P, MB, D_MODEL], dt.bfloat16, tag="wo")

            tri_sb = small.tile([P, P], dt.bfloat16, tag="tri")
            ones_sb = small.tile([1, DL], dt.bfloat16, tag="ones")
            nc.vector.memset(ones_sb, 1.0)
            # per-head ones column for the softmax denominator row
            # (psum partition bases must be 32-aligned: even heads use
            # row 64, odd heads row 0 — both in their zero region)
            vh = v_sb.rearrange("p b (g hh) c -> p b g hh c", hh=2)
            nc.vector.memset(vh[:, :, :, 0, HEAD_DIM:HEAD_DIM + 1], 1.0)
            nc.vector.memset(vh[:, :, :, 1, 0:1], 1.0)

            # ================= Stage B: projections + RoPE =================
            with (
                tc.tile_pool(name="bweights", bufs=1) as bweights,
                tc.tile_pool(name="bstage", bufs=2) as bstage,
                tc.tile_pool(name="proj_ps", bufs=2, space="PSUM") as proj_ps,
            ):
                xT_sb = bweights.tile([P, KC, S], dt.bfloat16, tag="xT")
                w_sbs = {}
                b_sbs = {}
                for nm in ("q", "k", "v"):
                    w_sbs[nm] = bweights.tile([P, KC, DL], dt.bfloat16,
                                              tag=f"w{nm}", name=f"w{nm}")
                cos_sb = bweights.tile([P, S], dt.bfloat16, tag="cos")
                sin_sb = bweights.tile([P, S], dt.bfloat16, tag="sin")
                for nm in ("q", "k"):
                    b_sbs[nm] = small.tile([P, MB], dt.float32,
                                           tag=f"b{nm}", name=f"b{nm}")
                b_sbs["v"] = small.tile([1, DL], dt.bfloat16, tag="bv",
                                        name="bv")

                # input DMAs, ordered so the first projection chains' data
                # (wq/wk m0 + xT) lands first; all on the sync HWDGE queue
                nc.sync.dma_start(out=b_sbs["q"], in_=bq_d)
                nc.sync.dma_start(out=b_sbs["k"], in_=bk_d)
                for kc in range(KC):
                    nc.sync.dma_start(out=w_sbs["q"][:, kc, :],
                                      in_=wq_d[kc * P:(kc + 1) * P, :])
                    nc.sync.dma_start(out=w_sbs["k"][:, kc, :],
                                      in_=wk_d[kc * P:(kc + 1) * P, :])
                    nc.sync.dma_start(out=xT_sb[:, kc, :],
                                      in_=xT_d[kc * P:(kc + 1) * P, :])
                nc.sync.dma_start(out=cos_sb, in_=cos_d)
                nc.sync.dma_start(out=sin_sb, in_=sin_d)
                for kc in range(KC):
                    nc.sync.dma_start(out=w_sbs["v"][:, kc, :],
                                      in_=wv_d[kc * P:(kc + 1) * P, :])
                nc.sync.dma_start(out=b_sbs["v"], in_=bv_d)
                nc.sync.dma_start(out=tri_sb, in_=tri_d)
                nc.sync.dma_start(
                    out=wo_sb, in_=wo_d.rearrange("(m p) n -> p m n", p=P))

                def emit_qk(nm, m):
                    # one head-pair projection: psum [128 dims, 2048 seq],
                    # 16 matmul chain (8 kc x 2 seq halves), then RoPE
                    w_sb, b_sb = w_sbs[nm], b_sbs[nm]
                    ps = proj_ps.tile([P, S], dt.float32, tag="proj")
                    for kc in range(KC):
                        for sh in range(4):
                            cs = slice(sh * DL, (sh + 1) * DL)
                            _tag(nc.tensor.matmul(
                                ps[:, cs],
                                lhsT=w_sb[:, kc, m * P:(m + 1) * P],
                                rhs=xT_sb[:, kc, cs],
                                start=(kc == 0), stop=(kc == KC - 1)),
                                "proj_qk")
                    raw = bstage.tile([P, S], dt.bfloat16, tag="raw")
                    nc.vector.tensor_scalar(raw, ps, b_sb[:, m:m + 1], None,
                                            ADD)
                    # rotate-half in the permuted basis: swap the 32-row
                    # halves of each head (sign lives in sinT)
                    qsw = bstage.tile([P, S], dt.bfloat16, tag="qsw")
                    for hh in range(2):
                        o32 = hh * HEAD_DIM
                        nc.sync.dma_start(
                            out=qsw[o32:o32 + HH, :],
                            in_=raw[o32 + HH:o32 + HEAD_DIM, :])
                        nc.sync.dma_start(
                            out=qsw[o32 + HH:o32 + HEAD_DIM, :],
                            in_=raw[o32:o32 + HH, :])
                    t1 = bstage.tile([P, S], dt.bfloat16, tag="t1")
                    nc.vector.tensor_mul(t1, raw, cos_sb)
                    t2 = bstage.tile([P, S], dt.bfloat16, tag="t2")
                    nc.gpsimd.tensor_mul(t2, qsw, sin_sb)
                    if nm == "q":
                        nc.vector.tensor_add(qTf[:, m, :], t1, t2)
                    else:
                        for hh in range(2):
                            po = hh * HEAD_DIM
                            nc.vector.tensor_add(
                                kTf[po:po + HEAD_DIM, 2 * m + hh, :],
                                t1[po:po + HEAD_DIM, :],
                                t2[po:po + HEAD_DIM, :])

                def emit_v(kb):
                    # v projection: natural [seq, dims] layout + ones col
                    ps = proj_ps.tile([P, DL], dt.float32, tag="proj",
                                      name="ps_v")
                    for kc in range(KC):
                        _tag(nc.tensor.matmul(
                            ps, lhsT=xT_sb[:, kc, kb * P:(kb + 1) * P],
                            rhs=w_sbs["v"][:, kc, :],
                            start=(kc == 0), stop=False), "proj_v")
                    _tag(nc.tensor.matmul(
                        ps, lhsT=ones_sb[:, :P], rhs=b_sbs["v"],
                        start=False, stop=True), "bias_v")
                    # even heads -> cols 0..63, odd heads -> cols 64..127
                    psh = ps.rearrange("p (g hh d) -> p g hh d", hh=2,
                                       d=HEAD_DIM)
                    vkb = v_sb.rearrange("p b (g hh) c -> p b g hh c",
                                         hh=2)[:, kb]
                    nc.vector.tensor_copy(
                        out=vkb[:, :, 0, 0:HEAD_DIM], in_=psh[:, :, 0, :])
                    nc.vector.tensor_copy(
                        out=vkb[:, :, 1, HEAD_DIM:P], in_=psh[:, :, 1, :])

                emit_qk("q", 0)
                emit_qk("k", 0)
                for kb in range(4):
                    emit_v(kb)
                emit_qk("q", 1)
                emit_qk("k", 1)
                for kb in range(4, 8):
                    emit_v(kb)
                emit_qk("q", 2)
                emit_qk("k", 2)
                emit_qk("q", 3)
                emit_qk("k", 3)
                for kb in range(8, NKB):
                    emit_v(kb)

            # ================= Stage C: attention per head =================
            with (
                tc.tile_pool(name="att_pool", bufs=2) as att_pool,
                tc.tile_pool(name="rtile", bufs=2) as rtile,
                tc.tile_pool(name="qk_ps", bufs=2, space="PSUM") as qk_ps,
                tc.tile_pool(name="yp_ps", bufs=2, space="PSUM") as yp_ps,
                tc.tile_pool(name="osb", bufs=2) as osb,
            ):
                def emit_c1_steps(h, att):
                    # QK^T + exp (+ causal mask on the diagonal block),
                    # yields once per ck
                    kh = kTf[:, h, :]
                    qh = qTf[:, h // 2, :]
                    for ck in range(NKB):
                        w = S - ck * P
                        base = ck * P
                        off = ATT_OFF[ck]
                        for s0 in range(0, w, CW):
                            sw = min(CW, w - s0)
                            ps = qk_ps.tile([P, CW], dt.float32, tag="qk",
                                            name="ps_qk")
                            for u0 in range(0, sw, DL):
                                uw = min(DL, sw - u0)
                                _tag(nc.tensor.matmul(
                                    ps[:, u0:u0 + uw],
                                    lhsT=kh[:, ck * P:(ck + 1) * P],
                                    rhs=qh[:, base + s0 + u0:
                                            base + s0 + u0 + uw],
                                    start=True, stop=True), "qk")
                            nc.scalar.activation(
                                out=att[:, off + s0:off + s0 + sw],
                                in_=ps[:, 0:sw], func=FExp, scale=SCALE)
                        nc.gpsimd.tensor_mul(
                            att[:, off:off + P], att[:, off:off + P], tri_sb)
                        yield

                def emit_c2_steps(h, att, on_half_done):
                    # flipped AV: lhsT = v (stationary), rhs = att cols
                    # (streaming); psum yp [128, 1024]: even heads rows
                    # 0-63 = y^T + denom row 64; odd heads denom row 0 +
                    # y^T rows 64-127.  Per q-half.
                    m, po = h // 2, (h % 2) * HEAD_DIM
                    dr = HEAD_DIM if h % 2 == 0 else 0
                    ys = slice(po, po + HEAD_DIM)
                    for half in range(2):
                        q0 = half * SH
                        yp = yp_ps.tile([P, SH], dt.float32, tag="yp",
                                        name="yp")
                        # psum accumulation groups are bank-granular: one
                        # clean chain per 512-col bank over its key blocks
                        for bank in range(SH // DL):
                            b0 = q0 + bank * DL          # first q col
                            ck_hi = (b0 + DL) // P       # blocks touching
                            for ck in range(ck_hi):
                                bs = ck * P
                                c0 = max(bs, b0)
                                a0 = ATT_OFF[ck] + c0 - bs
                                _tag(nc.tensor.matmul(
                                    yp[:, c0 - q0:b0 - q0 + DL],
                                    lhsT=v_sb[:, ck, h, :],
                                    rhs=att[:, a0:a0 + b0 + DL - c0],
                                    start=(ck == 0), stop=(ck == ck_hi - 1)),
                                    "av")
                                yield
                        # normalize: recip of denom row into a base-0 tile
                        # (cross-base DVE read from psum), base-0
                        # partition_broadcast, then fused scale+copy
                        # psum->sbuf (mixed psum/sbuf bases are legal)
                        rt = rtile.tile([HEAD_DIM, SH], dt.float32, tag="rt",
                                        name="rt")
                        nc.vector.reciprocal(
                            rt[0:1, :], yp[dr:dr + 1, :])
                        nc.gpsimd.partition_broadcast(
                            rt[:, :], rt[0:1, :])
                        nc.vector.tensor_mul(
                            yT_all[ys, m, q0:q0 + SH],
                            yp[ys, :], rt[:, :])
                        on_half_done(h, half)
                        yield

                def emit_oproj(qb):
                    # out-projection row block: psum [128 q, 1024],
                    # accumulate over head pairs; reuses qk_ps slots
                    ps = qk_ps.tile([P, D_MODEL], dt.float32, tag="qk",
                                    name="ps_o")
                    for t in range(2):
                        for m in range(MB):
                            _tag(nc.tensor.matmul(
                                ps[:, t * DL:(t + 1) * DL],
                                lhsT=yT_all[:, m, qb * P:(qb + 1) * P],
                                rhs=wo_sb[:, m, t * DL:(t + 1) * DL],
                                start=(m == 0), stop=(m == MB - 1)), "oproj")
                    ob = osb.tile([P, D_MODEL], dt.bfloat16, tag="ob")
                    nc.vector.tensor_copy(out=ob, in_=ps)
                    nc.sync.dma_start(
                        out=o_d[qb * P:(qb + 1) * P, :], in_=ob)

                def on_half_done(h, half):
                    # once the LAST head finishes a q-half, its out-proj
                    # row blocks are fully determined: chase them
                    if h == HPC - 1:
                        for qb in range(half * 8, half * 8 + 8):
                            emit_oproj(qb)

                att_tiles = {}
                att_tiles[0] = att_pool.tile([P, ATT_TOT], dt.bfloat16,
                                             tag="att", name="att0")
                for _ in emit_c1_steps(0, att_tiles[0]):
                    pass
                for h in range(1, HPC):
                    att_tiles[h] = att_pool.tile([P, ATT_TOT], dt.bfloat16,
                                                 tag="att", name=f"att{h}")
                    c1 = emit_c1_steps(h, att_tiles[h])
                    c2 = emit_c2_steps(h - 1, att_tiles[h - 1], on_half_done)
                    alive = True
                    while alive:
                        alive = False
                        if next(c1, "done") != "done":
                            alive = True
                        if next(c2, "done") != "done":
                            alive = True
                for _ in emit_c2_steps(HPC - 1, att_tiles[HPC - 1],
                                       on_half_done):
                    pass

    nc.compile()
    return nc


def _perm64():
    # pi: permuted-basis index j -> original head dim (evens then odds)
    return np.concatenate([np.arange(0, HEAD_DIM, 2),
                           np.arange(1, HEAD_DIM, 2)])


def _host_tables():
    pos = np.arange(S, dtype=np.float32)
    freq = np.arange(0, HEAD_DIM, 2, dtype=np.float32) / HEAD_DIM
    inv_freq = 1.0 / (10000.0 ** freq)
    ang = np.outer(inv_freq, pos)                       # [32, S]
    cos1 = np.cos(ang)
    sin1 = np.sin(ang)
    # pi-basis per-head tables [64, S]: rows 0..31 evens, 32..63 odds
    cosh = np.concatenate([cos1, cos1], axis=0)
    sinh = np.concatenate([-sin1, sin1], axis=0)        # sign folded in
    cosT = np.tile(cosh, (2, 1))                        # [128, S] head pair
    sinT = np.tile(sinh, (2, 1))
    tri = np.triu(np.ones((P, P), np.float32))          # keep k<=q in [k,q]
    return cosT.astype(BF16), sinT.astype(BF16), tri.astype(BF16)


def kernel(x, Wq, bq, Wk, bk, Wv, bv, Wo, bo):
    from concourse.bass_utils import run_bass_kernel_spmd

    x = np.asarray(x, np.float32)
    Wq, Wk, Wv, Wo = (np.asarray(a, np.float32) for a in (Wq, Wk, Wv, Wo))
    bq, bk, bv, bo = (np.asarray(a, np.float32) for a in (bq, bk, bv, bo))

    if "nc" not in _CACHE:
        _CACHE["nc"] = _build_bass()
    nc = _CACHE["nc"]

    cosT, sinT, tri = _host_tables()
    consts = {"cosT": cosT, "sinT": sinT, "tri": tri}

    # pi-basis permutation of q/k projection columns (per head)
    pi = _perm64()
    colperm = np.concatenate([h * HEAD_DIM + pi for h in range(N_HEADS)])
    Wq_p = Wq[:, colperm]
    Wk_p = Wk[:, colperm]
    bq_p = bq[colperm]
    bk_p = bk[colperm]

    xTs = [np.ascontiguousarray(x[b].T).astype(BF16) for b in range(B)]
    in_maps = []
    for c in range(N_CORES):
        b, g = c // HG, c % HG
        sl = slice(g * DL, (g + 1) * DL)
        in_maps.append({
            "xT": xTs[b],
            "wq": np.ascontiguousarray(Wq_p[:, sl]).astype(BF16),
            "wk": np.ascontiguousarray(Wk_p[:, sl]).astype(BF16),
            "wv": np.ascontiguousarray(Wv[:, sl]).astype(BF16),
            "wo": np.ascontiguousarray(Wo[sl, :]).astype(BF16),
            "bqT": np.ascontiguousarray(
                bq_p[sl].reshape(MB, P).T).astype(np.float32),
            "bkT": np.ascontiguousarray(
                bk_p[sl].reshape(MB, P).T).astype(np.float32),
            "bv": bv[sl].reshape(1, DL).astype(BF16),
            **consts,
        })

    res = run_bass_kernel_spmd(nc, in_maps, core_ids=list(range(N_CORES)))
    _CACHE["last_result"] = res
    out = np.empty((B, S, D_MODEL), np.float32)
    for b in range(B):
        out[b] = (res.results[HG * b]["o"].astype(np.float32) +
                  res.results[HG * b + 1]["o"].astype(np.float32))
    out += bo.astype(np.float32)
    return out


# revision 19
# speedup vs baseline: 1.3098x; 1.3098x over previous
"""Multi-head self-attention (RoPE, causal) Trainium2 Bass kernel.

Sharding: 8 cores = 4 batches x 2 head-groups (8 heads each).
Per core the device kernel computes, for its batch b and head-group g:
    q/k/v = x_b @ W*[:, g] (+bias), RoPE on q/k, causal softmax attention,
    partial out-projection y @ Wo[g]  -> [2048, 1024] (bf16).
Host sums the two head-group partials per batch and adds bo.

RoPE runs in a per-head permuted basis (evens then odds) prepared on the
host by permuting Wq/Wk columns: rotate-half becomes a swap of contiguous
32-partition halves (SBUF->SBUF DMAs) with the signs folded into the
sin table.  Scores are invariant to the shared q/k basis permutation.

Device layouts (per core):
    xT   [128, 8, 2048] bf16  x_b transposed (host-prepped)
    qT'  [128, 4, 2048] bf16  roped queries, head-pair dims on partitions
    kT'  [128, 8, 2048] bf16  roped keys, per head zero-padded to 128 rows
    v    [128, 16kb, 8h, 128] bf16; even heads: dims at cols 0-63, ones
                              col 64; odd heads: ones col 0, dims at
                              cols 64-127 (psum rows then match yT rows)
    att  [128, 17408] bf16    exp(scores^T) per head, causal-trapezoid packed
    yp   [128, 1024] psum     flipped AV: lhsT=v (stationary), rhs=att
                              (streaming) -> yT directly + denom row
    yT   [128, 4, 2048] bf16  normalized attention outputs for out-proj
"""

import os
import sys

import numpy as np

for _p in ("/opt/trn_rl_repo", "/root/.axon_site/_ro/trn_rl_repo"):
    if os.path.isdir(_p) and _p not in sys.path:
        sys.path.append(_p)

import ml_dtypes  # noqa: E402

BF16 = ml_dtypes.bfloat16

B, S, D_MODEL = 4, 2048, 1024
N_HEADS, HEAD_DIM = 16, 64
N_CORES = 8
HG = 2                      # head groups
HPC = N_HEADS // HG         # heads per core = 8
DL = HPC * HEAD_DIM         # local dims per core = 512
SCALE = HEAD_DIM ** -0.5
P = 128
KC = D_MODEL // P           # k chunks in projections = 8
MB = DL // P                # m blocks (head pairs) = 4
NKB = S // P                # 128-row blocks of sequence = 16
HH = HEAD_DIM // 2          # 32
SH = S // 2                 # 1024, attention processed in q halves
CW = 1024                   # score psum chunk width

# packed causal-trapezoid offsets: att row-block ck covers q in [128*ck, S)
ATT_OFF = [0] * (NKB + 1)
for _ck in range(NKB):
    ATT_OFF[_ck + 1] = ATT_OFF[_ck] + (S - P * _ck)
ATT_TOT = ATT_OFF[NKB]      # 17408

_CACHE = {}
STAGE_OF = {}


def _tag(inst, stage):
    try:
        STAGE_OF[str(inst.ins.name)] = stage
    except Exception:
        pass
    return inst


def _build_bass():
    import concourse.tile as tile
    from concourse import bacc, mybir

    dt = mybir.dt
    nc = bacc.Bacc("TRN2", target_bir_lowering=False, debug=False)

    def din(name, shape, d=dt.bfloat16):
        return nc.dram_tensor(name, shape, d, kind="ExternalInput").ap()

    xT_d = din("xT", [D_MODEL, S])
    wq_d = din("wq", [D_MODEL, DL])
    wk_d = din("wk", [D_MODEL, DL])
    wv_d = din("wv", [D_MODEL, DL])
    wo_d = din("wo", [DL, D_MODEL])
    bq_d = din("bqT", [P, MB], dt.float32)
    bk_d = din("bkT", [P, MB], dt.float32)
    bv_d = din("bv", [1, DL])
    cos_d = din("cosT", [P, S])
    sin_d = din("sinT", [P, S])          # sign-folded (pi-basis)
    msku_d = din("msku", [P, P])
    ident_d = din("ident", [P, P])
    o_d = nc.dram_tensor("o", [S, D_MODEL], dt.bfloat16,
                         kind="ExternalOutput").ap()

    FExp = mybir.ActivationFunctionType.Exp
    MUL = mybir.AluOpType.mult
    ADD = mybir.AluOpType.add

    with tile.TileContext(nc) as tc:
        with (
            tc.tile_pool(name="persist", bufs=1) as persist,
            tc.tile_pool(name="small", bufs=1) as small,
        ):
            qTf = persist.tile([P, MB, S], dt.bfloat16, tag="qTf")
            kTf = persist.tile([P, HPC, S], dt.bfloat16, tag="kTf")
            nc.vector.memset(kTf, 0.0)
            v_sb = persist.tile([P, NKB, HPC, P], dt.bfloat16, tag="v_sb")
            nc.vector.memset(v_sb, 0.0)
            yT_all = persist.tile([P, MB, S], dt.bfloat16, tag="yT")
            wo_sb = persist.tile([P, MB, D_MODEL], dt.bfloat16, tag="wo")

            msku_sb = small.tile([P, P], dt.bfloat16, tag="msku")
            ident_sb = small.tile([P, P], dt.bfloat16, tag="ident")
            ones_sb = small.tile([1, DL], dt.bfloat16, tag="ones")
            nc.vector.memset(ones_sb, 1.0)
            # per-head ones column for the softmax denominator row
            # (psum partition bases must be 32-aligned: even heads use
            # row 64, odd heads row 0 — both in their zero region)
            vh = v_sb.rearrange("p b (g hh) c -> p b g hh c", hh=2)
            nc.vector.memset(vh[:, :, :, 0, HEAD_DIM:HEAD_DIM + 1], 1.0)
            nc.vector.memset(vh[:, :, :, 1, 0:1], 1.0)

            # ================= Stage B: projections + RoPE =================
            with (
                tc.tile_pool(name="bweights", bufs=1) as bweights,
                tc.tile_pool(name="bstage", bufs=2) as bstage,
                tc.tile_pool(name="proj_ps", bufs=2, space="PSUM") as proj_ps,
            ):
                xT_sb = bweights.tile([P, KC, S], dt.bfloat16, tag="xT")
                w_sbs = {}
                b_sbs = {}
                for nm in ("q", "k", "v"):
                    w_sbs[nm] = bweights.tile([P, KC, DL], dt.bfloat16,
                                              tag=f"w{nm}", name=f"w{nm}")
                cos_sb = bweights.tile([P, S], dt.bfloat16, tag="cos")
                sin_sb = bweights.tile([P, S], dt.bfloat16, tag="sin")
                for nm in ("q", "k"):
                    b_sbs[nm] = small.tile([P, MB], dt.float32,
                                           tag=f"b{nm}", name=f"b{nm}")
                b_sbs["v"] = small.tile([1, DL], dt.bfloat16, tag="bv",
                                        name="bv")

                # input DMAs, ordered so the first projection chains' data
                # (wq/wk m0 + xT) lands first; all on the sync HWDGE queue
                nc.sync.dma_start(out=b_sbs["q"], in_=bq_d)
                nc.sync.dma_start(out=b_sbs["k"], in_=bk_d)
                for kc in range(KC):
                    nc.sync.dma_start(out=w_sbs["q"][:, kc, :],
                                      in_=wq_d[kc * P:(kc + 1) * P, :])
                    nc.sync.dma_start(out=w_sbs["k"][:, kc, :],
                                      in_=wk_d[kc * P:(kc + 1) * P, :])
                    nc.sync.dma_start(out=xT_sb[:, kc, :],
                                      in_=xT_d[kc * P:(kc + 1) * P, :])
                nc.sync.dma_start(out=cos_sb, in_=cos_d)
                nc.sync.dma_start(out=sin_sb, in_=sin_d)
                for kc in range(KC):
                    nc.sync.dma_start(out=w_sbs["v"][:, kc, :],
                                      in_=wv_d[kc * P:(kc + 1) * P, :])
                nc.sync.dma_start(out=b_sbs["v"], in_=bv_d)
                nc.sync.dma_start(out=msku_sb, in_=msku_d)
                nc.sync.dma_start(out=ident_sb, in_=ident_d)
                nc.sync.dma_start(
                    out=wo_sb, in_=wo_d.rearrange("(m p) n -> p m n", p=P))

                def emit_qk(nm, m):
                    # one head-pair projection: psum [128 dims, 2048 seq],
                    # 16 matmul chain (8 kc x 2 seq halves), then RoPE
                    w_sb, b_sb = w_sbs[nm], b_sbs[nm]
                    ps = proj_ps.tile([P, S], dt.float32, tag="proj")
                    for kc in range(KC):
                        for sh in range(4):
                            cs = slice(sh * DL, (sh + 1) * DL)
                            _tag(nc.tensor.matmul(
                                ps[:, cs],
                                lhsT=w_sb[:, kc, m * P:(m + 1) * P],
                                rhs=xT_sb[:, kc, cs],
                                start=(kc == 0), stop=(kc == KC - 1)),
                                "proj_qk")
                    raw = bstage.tile([P, S], dt.bfloat16, tag="raw")
                    nc.vector.tensor_scalar(raw, ps, b_sb[:, m:m + 1], None,
                                            ADD)
                    # rotate-half in the permuted basis: swap the 32-row
                    # halves of each head (sign lives in sinT)
                    qsw = bstage.tile([P, S], dt.bfloat16, tag="qsw")
                    for hh in range(2):
                        o32 = hh * HEAD_DIM
                        nc.sync.dma_start(
                            out=qsw[o32:o32 + HH, :],
                            in_=raw[o32 + HH:o32 + HEAD_DIM, :])
                        nc.sync.dma_start(
                            out=qsw[o32 + HH:o32 + HEAD_DIM, :],
                            in_=raw[o32:o32 + HH, :])
                    t1 = bstage.tile([P, S], dt.bfloat16, tag="t1")
                    nc.vector.tensor_mul(t1, raw, cos_sb)
                    t2 = bstage.tile([P, S], dt.bfloat16, tag="t2")
                    nc.gpsimd.tensor_mul(t2, qsw, sin_sb)
                    if nm == "q":
                        nc.vector.tensor_add(qTf[:, m, :], t1, t2)
                    else:
                        for hh in range(2):
                            po = hh * HEAD_DIM
                            nc.vector.tensor_add(
                                kTf[po:po + HEAD_DIM, 2 * m + hh, :],
                                t1[po:po + HEAD_DIM, :],
                                t2[po:po + HEAD_DIM, :])

                def emit_v(kb):
                    # v projection: natural [seq, dims] layout + ones col
                    ps = proj_ps.tile([P, DL], dt.float32, tag="proj",
                                      name="ps_v")
                    for kc in range(KC):
                        _tag(nc.tensor.matmul(
                            ps, lhsT=xT_sb[:, kc, kb * P:(kb + 1) * P],
                            rhs=w_sbs["v"][:, kc, :],
                            start=(kc == 0), stop=False), "proj_v")
                    _tag(nc.tensor.matmul(
                        ps, lhsT=ones_sb[:, :P], rhs=b_sbs["v"],
                        start=False, stop=True), "bias_v")
                    # even heads -> cols 0..63, odd heads -> cols 64..127
                    psh = ps.rearrange("p (g hh d) -> p g hh d", hh=2,
                                       d=HEAD_DIM)
                    vkb = v_sb.rearrange("p b (g hh) c -> p b g hh c",
                                         hh=2)[:, kb]
                    nc.vector.tensor_copy(
                        out=vkb[:, :, 0, 0:HEAD_DIM], in_=psh[:, :, 0, :])
                    nc.vector.tensor_copy(
                        out=vkb[:, :, 1, HEAD_DIM:P], in_=psh[:, :, 1, :])

                emit_qk("q", 0)
                emit_qk("k", 0)
                for kb in range(4):
                    emit_v(kb)
                emit_qk("q", 1)
                emit_qk("k", 1)
                for kb in range(4, 8):
                    emit_v(kb)
                emit_qk("q", 2)
                emit_qk("k", 2)
                emit_qk("q", 3)
                emit_qk("k", 3)
                for kb in range(8, NKB):
                    emit_v(kb)

            # ================= Stage C: attention per head =================
            with (
                tc.tile_pool(name="att_pool", bufs=2) as att_pool,
                tc.tile_pool(name="rtile", bufs=2) as rtile,
                tc.tile_pool(name="qk_ps", bufs=2, space="PSUM") as qk_ps,
                tc.tile_pool(name="yp_ps", bufs=2, space="PSUM") as yp_ps,
                tc.tile_pool(name="osb", bufs=2) as osb,
            ):
                def emit_c1_steps(h, att):
                    # QK^T + exp (+ causal mask on the diagonal block),
                    # yields once per ck
                    kh = kTf[:, h, :]
                    qh = qTf[:, h // 2, :]
                    for ck in range(NKB):
                        w = S - ck * P
                        base = ck * P
                        off = ATT_OFF[ck]
                        for s0 in range(0, w, CW):
                            sw = min(CW, w - s0)
                            ps = qk_ps.tile([P, CW], dt.float32, tag="qk",
                                            name="ps_qk")
                            for u0 in range(0, sw, DL):
                                uw = min(DL, sw - u0)
                                diag = (s0 == 0 and u0 == 0)
                                _tag(nc.tensor.matmul(
                                    ps[:, u0:u0 + uw],
                                    lhsT=kh[:, ck * P:(ck + 1) * P],
                                    rhs=qh[:, base + s0 + u0:
                                            base + s0 + u0 + uw],
                                    start=True, stop=not diag), "qk")
                                if diag:
                                    # causal mask: accumulate -8000 onto
                                    # strictly-masked diag entries so exp
                                    # flushes them to zero
                                    _tag(nc.tensor.matmul(
                                        ps[:, 0:P], lhsT=msku_sb,
                                        rhs=ident_sb,
                                        start=False, stop=True), "mask")
                            nc.scalar.activation(
                                out=att[:, off + s0:off + s0 + sw],
                                in_=ps[:, 0:sw], func=FExp, scale=SCALE)
                        yield

                def emit_c2_steps(h, att, on_half_done):
                    # flipped AV: lhsT = v (stationary), rhs = att cols
                    # (streaming); psum yp [128, 1024]: even heads rows
                    # 0-63 = y^T + denom row 64; odd heads denom row 0 +
                    # y^T rows 64-127.  Per q-half.
                    m, po = h // 2, (h % 2) * HEAD_DIM
                    dr = HEAD_DIM if h % 2 == 0 else 0
                    ys = slice(po, po + HEAD_DIM)
                    for half in range(2):
                        q0 = half * SH
                        yp = yp_ps.tile([P, SH], dt.float32, tag="yp",
                                        name="yp")
                        # psum accumulation groups are bank-granular: one
                        # clean chain per 512-col bank over its key blocks
                        for bank in range(SH // DL):
                            b0 = q0 + bank * DL          # first q col
                            ck_hi = (b0 + DL) // P       # blocks touching
                            for ck in range(ck_hi):
                                bs = ck * P
                                c0 = max(bs, b0)
                                a0 = ATT_OFF[ck] + c0 - bs
                                _tag(nc.tensor.matmul(
                                    yp[:, c0 - q0:b0 - q0 + DL],
                                    lhsT=v_sb[:, ck, h, :],
                                    rhs=att[:, a0:a0 + b0 + DL - c0],
                                    start=(ck == 0), stop=(ck == ck_hi - 1)),
                                    "av")
                                yield
                        # normalize: recip of denom row into a base-0 tile
                        # (cross-base DVE read from psum), base-0
                        # partition_broadcast, then fused scale+copy
                        # psum->sbuf (mixed psum/sbuf bases are legal)
                        rt = rtile.tile([HEAD_DIM, SH], dt.float32, tag="rt",
                                        name="rt")
                        nc.vector.reciprocal(
                            rt[0:1, :], yp[dr:dr + 1, :])
                        nc.gpsimd.partition_broadcast(
                            rt[:, :], rt[0:1, :])
                        nc.vector.tensor_mul(
                            yT_all[ys, m, q0:q0 + SH],
                            yp[ys, :], rt[:, :])
                        on_half_done(h, half)
                        yield

                def emit_oproj(qb):
                    # out-projection row block: psum [128 q, 1024],
                    # accumulate over head pairs; reuses qk_ps slots
                    ps = qk_ps.tile([P, D_MODEL], dt.float32, tag="qk",
                                    name="ps_o")
                    for t in range(2):
                        for m in range(MB):
                            _tag(nc.tensor.matmul(
                                ps[:, t * DL:(t + 1) * DL],
                                lhsT=yT_all[:, m, qb * P:(qb + 1) * P],
                                rhs=wo_sb[:, m, t * DL:(t + 1) * DL],
                                start=(m == 0), stop=(m == MB - 1)), "oproj")
                    ob = osb.tile([P, D_MODEL], dt.bfloat16, tag="ob")
                    nc.vector.tensor_copy(out=ob, in_=ps)
                    nc.sync.dma_start(
                        out=o_d[qb * P:(qb + 1) * P, :], in_=ob)

                def on_half_done(h, half):
                    # once the LAST head finishes a q-half, its out-proj
                    # row blocks are fully determined: chase them
                    if h == HPC - 1:
                        for qb in range(half * 8, half * 8 + 8):
                            emit_oproj(qb)

                att_tiles = {}
                att_tiles[0] = att_pool.tile([P, ATT_TOT], dt.bfloat16,
                                             tag="att", name="att0")
                for _ in emit_c1_steps(0, att_tiles[0]):
                    pass
                for h in range(1, HPC):
                    att_tiles[h] = att_pool.tile([P, ATT_TOT], dt.bfloat16,
                                                 tag="att", name=f"att{h}")
                    c1 = emit_c1_steps(h, att_tiles[h])
                    c2 = emit_c2_steps(h - 1, att_tiles[h - 1], on_half_done)
                    alive = True
                    while alive:
                        alive = False
                        if next(c1, "done") != "done":
                            alive = True
                        if next(c2, "done") != "done":
                            alive = True
                for _ in emit_c2_steps(HPC - 1, att_tiles[HPC - 1],
                                       on_half_done):
                    pass

    nc.compile()
    return nc


def _perm64():
    # pi: permuted-basis index j -> original head dim (evens then odds)
    return np.concatenate([np.arange(0, HEAD_DIM, 2),
                           np.arange(1, HEAD_DIM, 2)])


def _host_tables():
    pos = np.arange(S, dtype=np.float32)
    freq = np.arange(0, HEAD_DIM, 2, dtype=np.float32) / HEAD_DIM
    inv_freq = 1.0 / (10000.0 ** freq)
    ang = np.outer(inv_freq, pos)                       # [32, S]
    cos1 = np.cos(ang)
    sin1 = np.sin(ang)
    # pi-basis per-head tables [64, S]: rows 0..31 evens, 32..63 odds
    cosh = np.concatenate([cos1, cos1], axis=0)
    sinh = np.concatenate([-sin1, sin1], axis=0)        # sign folded in
    cosT = np.tile(cosh, (2, 1))                        # [128, S] head pair
    sinT = np.tile(sinh, (2, 1))
    # msku[p, c] = -8000 where c > p: via rhs=ident this adds -8000 to
    # score[k, q] for k > q (strictly-masked entries) before exp
    msku = -8000.0 * np.triu(np.ones((P, P), np.float32), 1)
    ident = np.eye(P, dtype=np.float32)
    return cosT.astype(BF16), sinT.astype(BF16), msku.astype(BF16), \
        ident.astype(BF16)


def kernel(x, Wq, bq, Wk, bk, Wv, bv, Wo, bo):
    from concourse.bass_utils import run_bass_kernel_spmd

    x = np.asarray(x, np.float32)
    Wq, Wk, Wv, Wo = (np.asarray(a, np.float32) for a in (Wq, Wk, Wv, Wo))
    bq, bk, bv, bo = (np.asarray(a, np.float32) for a in (bq, bk, bv, bo))

    if "nc" not in _CACHE:
        _CACHE["nc"] = _build_bass()
    nc = _CACHE["nc"]

    cosT, sinT, msku, ident = _host_tables()
    consts = {"cosT": cosT, "sinT": sinT, "msku": msku, "ident": ident}

    # pi-basis permutation of q/k projection columns (per head)
    pi = _perm64()
    colperm = np.concatenate([h * HEAD_DIM + pi for h in range(N_HEADS)])
    Wq_p = Wq[:, colperm]
    Wk_p = Wk[:, colperm]
    bq_p = bq[colperm]
    bk_p = bk[colperm]

    xTs = [np.ascontiguousarray(x[b].T).astype(BF16) for b in range(B)]
    in_maps = []
    for c in range(N_CORES):
        b, g = c // HG, c % HG
        sl = slice(g * DL, (g + 1) * DL)
        in_maps.append({
            "xT": xTs[b],
            "wq": np.ascontiguousarray(Wq_p[:, sl]).astype(BF16),
            "wk": np.ascontiguousarray(Wk_p[:, sl]).astype(BF16),
            "wv": np.ascontiguousarray(Wv[:, sl]).astype(BF16),
            "wo": np.ascontiguousarray(Wo[sl, :]).astype(BF16),
            "bqT": np.ascontiguousarray(
                bq_p[sl].reshape(MB, P).T).astype(np.float32),
            "bkT": np.ascontiguousarray(
                bk_p[sl].reshape(MB, P).T).astype(np.float32),
            "bv": bv[sl].reshape(1, DL).astype(BF16),
            **consts,
        })

    res = run_bass_kernel_spmd(nc, in_maps, core_ids=list(range(N_CORES)))
    _CACHE["last_result"] = res
    out = np.empty((B, S, D_MODEL), np.float32)
    for b in range(B):
        out[b] = (res.results[HG * b]["o"].astype(np.float32) +
                  res.results[HG * b + 1]["o"].astype(np.float32))
    out += bo.astype(np.float32)
    return out
